# revision 29
# baseline (speedup 1.0000x reference)
# Trainium2 Bass kernel for nn_LNKillingRelu: out = where(kf<=0, x, x + kf*d)
#   d  = einsum('fkn,gf->gkn', x, W)                      (per batch)
#   kf = einsum('fkn,kl,fln->fn', x, G, d)  broadcast over k
# G (Killing Gram of sl(3)): G[0,0]=G[4,4]=12, G[0,4]=G[4,0]=-6,
#   G[1,3]=G[3,1]=G[2,6]=G[6,2]=G[5,7]=G[7,5]=6; with kf' = kf/6:
#   kf' = x0*(2d0-d4) + x4*(2d4-d0) + x1*d3 + x3*d1 + x2*d6 + x6*d2
#       + x5*d7 + x7*d5
#   out = x + relu(6*kf') * d
#
# v12 (177.5us on HW; DVE-bound, ~99% busy, all TT ops at clean 2x rates):
#  - x k-planes live on-device in TAU=(1,3,5,7,2,6,0,4) order: both
#    product ops then read dc/aux contiguously (the strided-AP row
#    bubbles moved off the hot ops), o2's x operand stays flat, and the
#    host un-permutes planes on the way out for free.
#  - lag-3 software pipeline: iter m runs matmuls_m; products/t1 of m-1;
#    og/o2/out-store of m-3; t2/kf/gate of m-2.  DVE stream has no
#    cross-engine round-trip waits left (gaps ~4us total).
#  - DVE diet (7 TT ops/iter): products in 2 ops -- host ships a 4-plane
#    aux tensor [x6, x2, 2x0-x4, 2x4-x0] so the (2,6)+(0,4) pair
#    products merge into one 2048-elem op with dc as the strided read.
#    t1 = p[4:8]+=p[0:4] on DVE (SWDGE accum-DMA was tried: 3x payload
#    through the 435GB/s SBUF fabric + 8-10us completion latency made it
#    net slower, plus one intermittent bad run).  1-wide gate on ScalarE,
#    og as ONE flat 4096-elem TT with stride-0 broadcast gate read
#    (keeps the 2x DVE mode), o2 as one flat TT.
#  - host rearranges x/aux/out chunk-contiguous [nch, F, ..., nt]:
#    each x chunk is ONE 4MB DMA (8KB rows), out stores contiguous;
#    aux in half-chunk tiles (2-buf rotation) loaded 2-3 iters ahead.
#    Issue order x0A, W, x0B, then by need time.
#  - iter-0 products split by PSUM half (part A = swap-pairs needs only
#    dc half A) so the kf chain starts ~3us earlier.
#  - rejected on measurement: SWDGE accum t1 (slower + one flaky run),
#    GpSimd t2/kf (+34us), per-ft iter-0 load split (stalled mmB_0).
#
# Sharding: data-parallel over batch B=8 -> one batch per NeuronCore.

import os
from contextlib import ExitStack

import numpy as np

import concourse.bass as bass
import concourse.mybir as mybir
import concourse.tile as tile
from concourse.bass_utils import run_bass_kernel_spmd

B, F, K, N = 8, 512, 8, 2048
P = 128
FT = F // P  # 4 channel tiles
KH = K // 2  # planes per PSUM half

f32 = mybir.dt.float32
f16 = mybir.dt.float16
Alu = mybir.AluOpType
ActF = mybir.ActivationFunctionType

# t1 mode: 0 = all DVE (default; SWDGE accum was net-slower and flaky)
T1_MODE = int(os.environ.get("V7_T1", "0"))
# t2/kf reduction ops on GpSimd instead of DVE (DVE is the bottleneck at
# ~99.7% busy; GpSimd idles but shares an SBUF port with the DVE)
T2_GPS = os.environ.get("V9_T2_GPS", "0") == "1"


def _ap(base, off_elems, dims):
    """Raw AP from a base AP: keep partition dim, replace free dims."""
    return bass.AP(
        tensor=base.tensor,
        offset=base.offset + off_elems,
        ap=[base.ap[0]] + dims,
    )


def _rap(base, off_elems, dims):
    """Fully raw AP (partition dim included in dims)."""
    return bass.AP(tensor=base.tensor, offset=off_elems, ap=dims)


def build_nc(n_total=N, nt=512):
    nch = n_total // nt
    KN = K * nt          # elems per (f row, chunk) = 4096
    nc = bass.Bass(detect_race_conditions=False)
    # chunk-contiguous layouts, host-rearranged
    xr = nc.dram_tensor("xr", [nch, F, K, nt], f16, kind="ExternalInput")
    wt = nc.dram_tensor("wt", [F, F], f16, kind="ExternalInput")  # W^T (f, g)
    auxr = nc.dram_tensor("auxr", [nch, F, 4, nt], f16, kind="ExternalInput")
    outr = nc.dram_tensor("outr", [nch, F, K, nt], f16, kind="ExternalOutput")
    xr_b = xr[:, :, :, :]
    auxr_b = auxr[:, :, :, :]
    outr_b = outr[:, :, :, :]

    with tile.TileContext(nc) as tc, ExitStack() as ctx:
        wpool = ctx.enter_context(tc.tile_pool(name="w", bufs=1))
        xpool = ctx.enter_context(tc.tile_pool(name="xc", bufs=3))
        axpool = ctx.enter_context(tc.tile_pool(name="ax", bufs=2))
        papool = ctx.enter_context(tc.tile_pool(name="pda", bufs=1, space="PSUM"))
        pbpool = ctx.enter_context(tc.tile_pool(name="pdb", bufs=1, space="PSUM"))
        dcpool = ctx.enter_context(tc.tile_pool(name="dc", bufs=4))
        prpool = ctx.enter_context(tc.tile_pool(name="prod", bufs=2))
        s2pool = ctx.enter_context(tc.tile_pool(name="s2", bufs=2))
        s3pool = ctx.enter_context(tc.tile_pool(name="s3", bufs=3))
        opool = ctx.enter_context(tc.tile_pool(name="og", bufs=2))

        # DMA issue order matters: the Sync HWDGE queue is FIFO and the
        # fabric is the bottleneck during ramp.  mmA_0 needs x0 half A + W.
        xt0 = xpool.tile([P, FT * KN], f16, tag="xc", name="x0")
        nc.sync.dma_start(
            out=_ap(xt0[:], 0, [[KN, FT], [1, KH * nt]]),
            in_=_rap(xr_b, 0, [[KN, P], [P * KN, FT], [1, KH * nt]]),
        )

        # resident W^T tiles: wsb[ft][p, g] , f = ft*128+p
        wsb = []
        for ft in range(FT):
            w_t = wpool.tile([P, F], f16, tag=f"w{ft}")
            nc.sync.dma_start(out=w_t[:], in_=wt[ft * P : (ft + 1) * P, :])
            wsb.append(w_t)

        # x0 half B
        nc.sync.dma_start(
            out=_ap(xt0[:], KH * nt, [[KN, FT], [1, KH * nt]]),
            in_=_rap(xr_b, KH * nt, [[KN, P], [P * KN, FT], [1, KH * nt]]),
        )

        axh = {}

        def load_aux(c, h):
            # aux4 planes per (f,chunk): [x6, x2, 2x0-x4, 2x4-x0] so the
            # B+C products merge into ONE 2048-elem op (dc is the strided
            # operand).  Half-chunk tiles (gt pair {2h, 2h+1}) in a 2-buf
            # rotation, loaded 2-3 iters ahead of first use.
            at = axpool.tile([P, 2 * 4 * nt], f16, tag="ax")
            nc.sync.dma_start(
                out=_ap(at[:], 0, [[4 * nt, 2], [1, 4 * nt]]),
                in_=_rap(
                    auxr_b,
                    (c * F + 2 * h * P) * 4 * nt,
                    [[4 * nt, P], [P * 4 * nt, 2], [1, 4 * nt]],
                ),
            )
            axh[(c, h)] = at
            return at

        # Walrus only allows ONE sync wait per Matmult (waits ride the
        # LDWEIGHTS struct).  Warmup matmuls make PE observe each W-DMA
        # semaphore individually so later matmuls never wait on W.
        # Before them: a dependency-free scratch train sized to bridge the
        # PE from preamble-end (~8us) to W-arrival (~14us) -- pstate
        # warmth decays within a few us of idle, so the train must run
        # CONTIGUOUSLY into the first real matmul for mmA_0 to be hot.
        scratch = wpool.tile([P, F], f16, tag="scratch")
        nc.gpsimd.memset(scratch[:], 0.0)
        warm = papool.tile([P, KH, nt], f32, tag="pda")
        for r in range(16):
            # 512-col like the real matmuls: the PE hot-path state is
            # config-specific (256-col warmups left mmA_0 at 634ns)
            nc.tensor.matmul(
                warm[:, 0, 0:512], scratch[:, 0:P], scratch[:, 0:512],
                start=True, stop=True,
            )
        for ft in range(FT):
            nc.tensor.matmul(
                warm[:, 0, 0:1], wsb[ft][:, 0:P], wsb[ft][:, 0:1], start=True, stop=True
            )

        def emit_products(st, part=2):
            # x planes are host-permuted to tau=(1,3,5,7,2,6,0,4); dc slot
            # j holds d_tau[j].  p slots: 0..3 <- x[0:4]*dc(1,0,3,2)
            # (the swap-pair products), 4..7 <- aux4*dc[4:8] (both flat).
            # part: 0 = A only (needs dc half A), 1 = BC only, 2 = both
            xt, xo, dc = st["xt"], st["xo"], st["dc"]
            if "p" in st:
                p = st["p"]
            else:
                p = prpool.tile([P, K, nt], f16, tag="p")
                st["p"] = p
            if part in (0, 2):
                nc.vector.tensor_tensor(
                    out=_ap(p[:], 0, [[1, 4 * nt]]),
                    in0=_ap(xt, xo, [[1, 4 * nt]]),
                    in1=_ap(dc[:], nt, [[2 * nt, 2], [-nt, 2], [1, nt]]),
                    op=Alu.mult,
                )
            if part in (1, 2):
                nc.vector.tensor_tensor(
                    out=_ap(p[:], 4 * nt, [[1, 4 * nt]]),
                    in0=st["ax"],
                    in1=_ap(dc[:], 4 * nt, [[1, 4 * nt]]),
                    op=Alu.mult,
                )

        def emit_t1(st):
            p = st["p"]
            if T1_MODE == 1:
                nc.gpsimd.dma_start(
                    out=p[:, 4:8, :], in_=p[:, 0:4, :], accum_op=Alu.add
                )
            elif T1_MODE == 3:
                # two 1024-elem accum DMAs (2x margin below the 2048 CCE
                # descriptor limit)
                nc.gpsimd.dma_start(
                    out=p[:, 4:6, :], in_=p[:, 0:2, :], accum_op=Alu.add
                )
                nc.gpsimd.dma_start(
                    out=p[:, 6:8, :], in_=p[:, 2:4, :], accum_op=Alu.add
                )
            else:
                nc.vector.tensor_tensor(
                    out=_ap(p[:], 4 * nt, [[1, 4 * nt]]),
                    in0=_ap(p[:], 0, [[1, 4 * nt]]),
                    in1=_ap(p[:], 4 * nt, [[1, 4 * nt]]),
                    op=Alu.add,
                )

        def emit_t2_kf(st):
            p = st["p"]
            eng = nc.gpsimd if T2_GPS else nc.vector
            t2 = s2pool.tile([P, 2, nt], f16, tag="t2")
            eng.tensor_tensor(
                out=_ap(t2[:], 0, [[1, 2 * nt]]),
                in0=_ap(p[:], 4 * nt, [[1, 2 * nt]]),
                in1=_ap(p[:], 6 * nt, [[1, 2 * nt]]),
                op=Alu.add,
            )
            kf = s3pool.tile([P, nt], f16, tag="kf", bufs=2)
            eng.tensor_tensor(
                out=kf[:], in0=t2[:, 0, :], in1=t2[:, 1, :], op=Alu.add
            )
            st["kf"] = kf

        def emit_gate(st):
            gate = s3pool.tile([P, nt], f16, tag="gate")
            nc.scalar.activation(
                out=gate[:], in_=st["kf"][:], func=ActF.Relu, scale=6.0
            )
            st["gate"] = gate

        def emit_og_o2(st):
            dc, xt, xo = st["dc"], st["xt"], st["xo"]
            og = opool.tile([P, K, nt], f16, tag="og", bufs=1)
            nc.vector.tensor_tensor(
                out=_ap(og[:], 0, [[1, K * nt]]),
                in0=_ap(dc[:], 0, [[1, K * nt]]),
                in1=_ap(st["gate"][:], 0, [[0, K], [1, nt]]),
                op=Alu.mult,
            )
            o2 = opool.tile([P, K, nt], f16, tag="o2")
            nc.vector.tensor_tensor(
                out=_ap(o2[:], 0, [[1, K * nt]]),
                in0=_ap(og[:], 0, [[1, K * nt]]),
                in1=_ap(xt, xo, [[1, K * nt]]),
                op=Alu.add,
            )
            st["o2"] = o2

        def emit_out(st):
            c, gt = st["c"], st["gt"]
            nc.scalar.dma_start(
                out=_rap(
                    outr_b,
                    (c * F + gt * P) * KN,
                    [[KN, P], [1, KN]],
                ),
                in_=st["o2"][:],
            )

        prev = None   # iter m-1: products + t1 pending
        prev2 = None  # iter m-2: t2/kf/gate pending
        prev3 = None  # iter m-3: og/o2/out pending

        for c in range(nch):
            # one x tile per chunk: [P, FT*K*nt] (f = ft*128 + p)
            if c == 0:
                xt = xt0
            else:
                xt = xpool.tile([P, FT * KN], f16, tag="xc")
                nc.sync.dma_start(
                    out=xt[:],
                    in_=_rap(
                        xr_b,
                        c * F * KN,
                        [[KN, P], [P * KN, FT], [1, KN]],
                    ),
                )
            if c == 0:
                load_aux(0, 0)
            for gt in range(FT):
                xo = gt * KN
                dc = dcpool.tile([P, K, nt], f16, tag="dc")
                cur = {
                    "dc": dc, "xt": xt[:], "xo": xo, "c": c, "gt": gt,
                    "ax": _ap(axh[(c, gt // 2)][:], (gt % 2) * 4 * nt,
                              [[1, 4 * nt]]),
                }

                # ScalarE head: dcB of the PREVIOUS iter
                if prev is not None:
                    nc.scalar.copy(out=prev["dc"][:, KH:K, :], in_=prev["pdB"][:])
                # ---- matmul halves -> PSUM ----
                pds = []
                for half, pool in ((0, papool), (1, pbpool)):
                    pd = pool.tile([P, KH, nt], f32, tag=("pda", "pdb")[half])
                    # Dummy matmul absorbs the PSUM-slot-release wait
                    # (1-wait limit on Matmult structs).
                    nc.tensor.matmul(
                        pd[:, 0, 0:1], wsb[0][:, 0:P], wsb[0][:, 0:1],
                        start=True, stop=True,
                    )
                    k0 = half * KH
                    for ft in range(FT):
                        for jj in range(KH):
                            nc.tensor.matmul(
                                pd[:, jj, :],
                                wsb[ft][:, gt * P : (gt + 1) * P],
                                _ap(xt[:], ft * KN + (k0 + jj) * nt, [[1, nt]]),
                                start=(ft == 0),
                                stop=(ft == FT - 1),
                            )
                    pds.append(pd)
                    if half == 0:
                        nc.scalar.copy(out=dc[:, 0:KH, :], in_=pd[:])
                        if prev is None and prev2 is None:
                            # iter 0: products A need only dc half A
                            emit_products(cur, part=0)
                cur["pdB"] = pds[1]

                # ---- elementwise streams (DVE order = emission order) ----
                if prev is not None:
                    emit_products(prev, part=2 if "p" not in prev else 1)
                    emit_t1(prev)
                if gt == 0:
                    load_aux(c, 1)
                elif gt == 2 and c + 1 < nch:
                    load_aux(c + 1, 0)
                if prev3 is not None:
                    emit_og_o2(prev3)
                if prev2 is not None:
                    emit_t2_kf(prev2)
                    emit_gate(prev2)
                if prev3 is not None:
                    emit_out(prev3)

                prev3 = prev2
                prev2 = prev
                prev = cur

        # ---- drain ----
        nc.scalar.copy(out=prev["dc"][:, KH:K, :], in_=prev["pdB"][:])
        emit_products(prev, part=1 if "p" in prev else 2)
        emit_t1(prev)
        emit_og_o2(prev3)
        emit_t2_kf(prev2)
        emit_gate(prev2)
        emit_out(prev3)
        emit_og_o2(prev2)
        emit_t2_kf(prev)
        emit_gate(prev)
        emit_out(prev2)
        emit_og_o2(prev)
        emit_out(prev)

    _split_waits(nc)
    return nc


# Engine datapath structs (Matmult/TT/STT/Act/...) only carry ONE sync wait on
# TRN2 walrus; sequencer instructions (NoOp) can each carry one more.  Hoist
# surplus waits onto same-engine NoOps placed just before the instruction.
def _split_waits(nc):
    nnop = 0
    for fn in nc.m.functions:
        for blk in fn.blocks:
            out = []
            for inst in blk.instructions:
                si = inst.sync_info
                if si is not None and si.on_wait and len(si.on_wait) > 1:
                    for w in si.on_wait[:-1]:
                        nop = mybir.InstNoOp(
                            name=f"{inst.name}-sw{nnop}",
                            opcode="NoOp",
                            engine=inst.engine,
                            sync_info=mybir.SyncInfo(on_wait=[w], on_update=[]),
                        )
                        nnop += 1
                        out.append(nop)
                    inst.sync_info = mybir.SyncInfo(
                        on_wait=[si.on_wait[-1]], on_update=list(si.on_update)
                    )
                out.append(inst)
            blk.instructions[:] = out
    return nc


_NC_CACHE = {}

NCH, NT = N // 512, 512


def _get_nc(n_total=N, nt=NT):
    key = (n_total, nt)
    if key not in _NC_CACHE:
        _NC_CACHE[key] = build_nc(n_total, nt)
    return _NC_CACHE[key]


def _to_f16(a: np.ndarray) -> np.ndarray:
    return np.ascontiguousarray(a.astype(np.float16))


def _chunked(a: np.ndarray) -> np.ndarray:
    # [F, C, N] -> chunk-contiguous [nch, F, C, nt]
    Fd, Cd, Nd = a.shape
    return np.ascontiguousarray(
        a.reshape(Fd, Cd, NCH, NT).transpose(2, 0, 1, 3)
    )


# x k-planes stored on-device in this order; matmul/products/out are all
# slot-relative, the host un-permutes planes on the way out (free).
TAU = [1, 3, 5, 7, 2, 6, 0, 4]
INV = [6, 0, 4, 1, 7, 2, 5, 3]  # INV[k] = slot of true plane k


def make_in_maps(x: np.ndarray, W: np.ndarray):
    wt = _to_f16(W.T.copy())
    x16 = _to_f16(x[:, :, TAU, :])
    xa = np.stack(
        [
            x[:, :, 6, :],
            x[:, :, 2, :],
            2.0 * x[:, :, 0, :] - x[:, :, 4, :],
            2.0 * x[:, :, 4, :] - x[:, :, 0, :],
        ],
        axis=2,
    )
    xa16 = _to_f16(xa)
    return [
        {"xr": _chunked(x16[b]), "wt": wt, "auxr": _chunked(xa16[b])}
        for b in range(B)
    ]


def post(res) -> np.ndarray:
    # gather + un-chunk: [nch, F, K, nt] -> [F, K, N]
    return np.stack(
        [
            res.results[b]["outr"]
            .transpose(1, 2, 0, 3)[:, INV]
            .reshape(F, K, N)
            .astype(np.float32)
            for b in range(B)
        ],
        axis=0,
    )


def kernel(x: np.ndarray, W: np.ndarray) -> np.ndarray:
    assert x.shape == (B, F, K, N) and W.shape == (F, F)
    in_maps = make_in_maps(x, W)
    nc = _get_nc()
    res = run_bass_kernel_spmd(nc, in_maps, list(range(B)))
    return post(res)


if __name__ == "__main__":
    xs = np.random.randn(B, F, K, N).astype(np.float32)
    Ws = (np.random.randn(F, F) / np.sqrt(F)).astype(np.float32)
    o = kernel(xs, Ws)
    print(o.shape, o.dtype)


# revision 31
# speedup vs baseline: 1.0188x; 1.0188x over previous
# Trainium2 Bass kernel for nn_LNKillingRelu: out = where(kf<=0, x, x + kf*d)
#   d  = einsum('fkn,gf->gkn', x, W)                      (per batch)
#   kf = einsum('fkn,kl,fln->fn', x, G, d)  broadcast over k
# G (Killing Gram of sl(3)): G[0,0]=G[4,4]=12, G[0,4]=G[4,0]=-6,
#   G[1,3]=G[3,1]=G[2,6]=G[6,2]=G[5,7]=G[7,5]=6; with kf' = kf/6:
#   kf' = x0*(2d0-d4) + x4*(2d4-d0) + x1*d3 + x3*d1 + x2*d6 + x6*d2
#       + x5*d7 + x7*d5
#   out = x + relu(6*kf') * d
#
# v12 (177.5us on HW; DVE-bound, ~99% busy, all TT ops at clean 2x rates):
#  - x k-planes live on-device in TAU=(1,3,5,7,2,6,0,4) order: both
#    product ops then read dc/aux contiguously (the strided-AP row
#    bubbles moved off the hot ops), o2's x operand stays flat, and the
#    host un-permutes planes on the way out for free.
#  - lag-3 software pipeline: iter m runs matmuls_m; products/t1 of m-1;
#    og/o2/out-store of m-3; t2/kf/gate of m-2.  DVE stream has no
#    cross-engine round-trip waits left (gaps ~4us total).
#  - DVE diet (7 TT ops/iter): products in 2 ops -- host ships a 4-plane
#    aux tensor [x6, x2, 2x0-x4, 2x4-x0] so the (2,6)+(0,4) pair
#    products merge into one 2048-elem op with dc as the strided read.
#    t1 = p[4:8]+=p[0:4] on DVE (SWDGE accum-DMA was tried: 3x payload
#    through the 435GB/s SBUF fabric + 8-10us completion latency made it
#    net slower, plus one intermittent bad run).  1-wide gate on ScalarE,
#    og as ONE flat 4096-elem TT with stride-0 broadcast gate read
#    (keeps the 2x DVE mode), o2 as one flat TT.
#  - host rearranges x/aux/out chunk-contiguous [nch, F, ..., nt]:
#    each x chunk is ONE 4MB DMA (8KB rows), out stores contiguous;
#    aux in half-chunk tiles (2-buf rotation) loaded 2-3 iters ahead.
#    Issue order x0A, W, x0B, then by need time.
#  - iter-0 products split by PSUM half (part A = swap-pairs needs only
#    dc half A) so the kf chain starts ~3us earlier.
#  - rejected on measurement: SWDGE accum t1 (slower + one flaky run),
#    GpSimd t2/kf (+34us), per-ft iter-0 load split (stalled mmB_0).
#
# Sharding: data-parallel over batch B=8 -> one batch per NeuronCore.

import os
from contextlib import ExitStack

import numpy as np

import concourse.bass as bass
import concourse.mybir as mybir
import concourse.tile as tile
from concourse.bass_utils import run_bass_kernel_spmd

B, F, K, N = 8, 512, 8, 2048
P = 128
FT = F // P  # 4 channel tiles
KH = K // 2  # planes per PSUM half

f32 = mybir.dt.float32
f16 = mybir.dt.float16
Alu = mybir.AluOpType
ActF = mybir.ActivationFunctionType

# t1 mode: 0 = all DVE (default; SWDGE accum was net-slower and flaky)
T1_MODE = int(os.environ.get("V7_T1", "0"))
# t2/kf reduction ops on GpSimd instead of DVE (DVE is the bottleneck at
# ~99.7% busy; GpSimd idles but shares an SBUF port with the DVE)
T2_GPS = os.environ.get("V9_T2_GPS", "0") == "1"


def _ap(base, off_elems, dims):
    """Raw AP from a base AP: keep partition dim, replace free dims."""
    return bass.AP(
        tensor=base.tensor,
        offset=base.offset + off_elems,
        ap=[base.ap[0]] + dims,
    )


def _rap(base, off_elems, dims):
    """Fully raw AP (partition dim included in dims)."""
    return bass.AP(tensor=base.tensor, offset=off_elems, ap=dims)


def build_nc(n_total=N, nt=512):
    nch = n_total // nt
    KN = K * nt          # elems per (f row, chunk) = 4096
    nc = bass.Bass(detect_race_conditions=False)
    # chunk-contiguous layouts, host-rearranged
    xr = nc.dram_tensor("xr", [nch, F, K, nt], f16, kind="ExternalInput")
    wt = nc.dram_tensor("wt", [F, F], f16, kind="ExternalInput")  # W^T (f, g)
    auxr = nc.dram_tensor("auxr", [nch, F, 4, nt], f16, kind="ExternalInput")
    outr = nc.dram_tensor("outr", [nch, F, K, nt], f16, kind="ExternalOutput")
    xr_b = xr[:, :, :, :]
    auxr_b = auxr[:, :, :, :]
    outr_b = outr[:, :, :, :]

    with tile.TileContext(nc) as tc, ExitStack() as ctx:
        wpool = ctx.enter_context(tc.tile_pool(name="w", bufs=1))
        xpool = ctx.enter_context(tc.tile_pool(name="xc", bufs=3))
        axpool = ctx.enter_context(tc.tile_pool(name="ax", bufs=2))
        papool = ctx.enter_context(tc.tile_pool(name="pda", bufs=1, space="PSUM"))
        pbpool = ctx.enter_context(tc.tile_pool(name="pdb", bufs=1, space="PSUM"))
        dcpool = ctx.enter_context(tc.tile_pool(name="dc", bufs=4))
        prpool = ctx.enter_context(tc.tile_pool(name="prod", bufs=2))
        s2pool = ctx.enter_context(tc.tile_pool(name="s2", bufs=2))
        s3pool = ctx.enter_context(tc.tile_pool(name="s3", bufs=3))
        opool = ctx.enter_context(tc.tile_pool(name="og", bufs=2))

        # DMA issue order matters: the Sync HWDGE queue is FIFO and the
        # fabric is the bottleneck during ramp.  mmA_0 needs x0 half A + W.
        xt0 = xpool.tile([P, FT * KN], f16, tag="xc", name="x0")
        nc.sync.dma_start(
            out=_ap(xt0[:], 0, [[KN, FT], [1, KH * nt]]),
            in_=_rap(xr_b, 0, [[KN, P], [P * KN, FT], [1, KH * nt]]),
        )

        # resident W^T tiles: wsb[ft][p, g] , f = ft*128+p
        wsb = []
        for ft in range(FT):
            w_t = wpool.tile([P, F], f16, tag=f"w{ft}")
            nc.sync.dma_start(out=w_t[:], in_=wt[ft * P : (ft + 1) * P, :])
            wsb.append(w_t)

        # x0 half B
        nc.sync.dma_start(
            out=_ap(xt0[:], KH * nt, [[KN, FT], [1, KH * nt]]),
            in_=_rap(xr_b, KH * nt, [[KN, P], [P * KN, FT], [1, KH * nt]]),
        )

        axh = {}

        def load_aux(c, h):
            # aux4 planes per (f,chunk): [x6, x2, 2x0-x4, 2x4-x0] so the
            # B+C products merge into ONE 2048-elem op (dc is the strided
            # operand).  Half-chunk tiles (gt pair {2h, 2h+1}) in a 2-buf
            # rotation, loaded 2-3 iters ahead of first use.
            at = axpool.tile([P, 2 * 4 * nt], f16, tag="ax")
            nc.sync.dma_start(
                out=_ap(at[:], 0, [[4 * nt, 2], [1, 4 * nt]]),
                in_=_rap(
                    auxr_b,
                    (c * F + 2 * h * P) * 4 * nt,
                    [[4 * nt, P], [P * 4 * nt, 2], [1, 4 * nt]],
                ),
            )
            axh[(c, h)] = at
            return at

        # Walrus only allows ONE sync wait per Matmult (waits ride the
        # LDWEIGHTS struct).  Warmup matmuls make PE observe each W-DMA
        # semaphore individually so later matmuls never wait on W.
        warm = papool.tile([P, KH, nt], f32, tag="pda")
        for ft in range(FT):
            nc.tensor.matmul(
                warm[:, 0, 0:1], wsb[ft][:, 0:P], wsb[ft][:, 0:1], start=True, stop=True
            )

        def emit_products(st, part=2):
            # x planes are host-permuted to tau=(1,3,5,7,2,6,0,4); dc slot
            # j holds d_tau[j].  p slots: 0..3 <- x[0:4]*dc(1,0,3,2)
            # (the swap-pair products), 4..7 <- aux4*dc[4:8] (both flat).
            # part: 0 = A only (needs dc half A), 1 = BC only, 2 = both
            xt, xo, dc = st["xt"], st["xo"], st["dc"]
            if "p" in st:
                p = st["p"]
            else:
                p = prpool.tile([P, K, nt], f16, tag="p")
                st["p"] = p
            if part in (0, 2):
                nc.vector.tensor_tensor(
                    out=_ap(p[:], 0, [[1, 4 * nt]]),
                    in0=_ap(xt, xo, [[1, 4 * nt]]),
                    in1=_ap(dc[:], nt, [[2 * nt, 2], [-nt, 2], [1, nt]]),
                    op=Alu.mult,
                )
            if part in (1, 2):
                nc.vector.tensor_tensor(
                    out=_ap(p[:], 4 * nt, [[1, 4 * nt]]),
                    in0=st["ax"],
                    in1=_ap(dc[:], 4 * nt, [[1, 4 * nt]]),
                    op=Alu.mult,
                )

        def emit_t1(st):
            p = st["p"]
            if T1_MODE == 1:
                nc.gpsimd.dma_start(
                    out=p[:, 4:8, :], in_=p[:, 0:4, :], accum_op=Alu.add
                )
            elif T1_MODE == 3:
                # two 1024-elem accum DMAs (2x margin below the 2048 CCE
                # descriptor limit)
                nc.gpsimd.dma_start(
                    out=p[:, 4:6, :], in_=p[:, 0:2, :], accum_op=Alu.add
                )
                nc.gpsimd.dma_start(
                    out=p[:, 6:8, :], in_=p[:, 2:4, :], accum_op=Alu.add
                )
            else:
                nc.vector.tensor_tensor(
                    out=_ap(p[:], 4 * nt, [[1, 4 * nt]]),
                    in0=_ap(p[:], 0, [[1, 4 * nt]]),
                    in1=_ap(p[:], 4 * nt, [[1, 4 * nt]]),
                    op=Alu.add,
                )

        def emit_t2_kf(st):
            p = st["p"]
            eng = nc.gpsimd if T2_GPS else nc.vector
            t2 = s2pool.tile([P, 2, nt], f16, tag="t2")
            eng.tensor_tensor(
                out=_ap(t2[:], 0, [[1, 2 * nt]]),
                in0=_ap(p[:], 4 * nt, [[1, 2 * nt]]),
                in1=_ap(p[:], 6 * nt, [[1, 2 * nt]]),
                op=Alu.add,
            )
            kf = s3pool.tile([P, nt], f16, tag="kf", bufs=2)
            eng.tensor_tensor(
                out=kf[:], in0=t2[:, 0, :], in1=t2[:, 1, :], op=Alu.add
            )
            st["kf"] = kf

        def emit_gate(st):
            gate = s3pool.tile([P, nt], f16, tag="gate")
            nc.scalar.activation(
                out=gate[:], in_=st["kf"][:], func=ActF.Relu, scale=6.0
            )
            st["gate"] = gate

        def emit_og_o2(st):
            dc, xt, xo = st["dc"], st["xt"], st["xo"]
            og = opool.tile([P, K, nt], f16, tag="og", bufs=1)
            nc.vector.tensor_tensor(
                out=_ap(og[:], 0, [[1, K * nt]]),
                in0=_ap(dc[:], 0, [[1, K * nt]]),
                in1=_ap(st["gate"][:], 0, [[0, K], [1, nt]]),
                op=Alu.mult,
            )
            o2 = opool.tile([P, K, nt], f16, tag="o2")
            nc.vector.tensor_tensor(
                out=_ap(o2[:], 0, [[1, K * nt]]),
                in0=_ap(og[:], 0, [[1, K * nt]]),
                in1=_ap(xt, xo, [[1, K * nt]]),
                op=Alu.add,
            )
            st["o2"] = o2

        def emit_out(st):
            c, gt = st["c"], st["gt"]
            nc.scalar.dma_start(
                out=_rap(
                    outr_b,
                    (c * F + gt * P) * KN,
                    [[KN, P], [1, KN]],
                ),
                in_=st["o2"][:],
            )

        prev = None   # iter m-1: products + t1 pending
        prev2 = None  # iter m-2: t2/kf/gate pending
        prev3 = None  # iter m-3: og/o2/out pending

        for c in range(nch):
            # one x tile per chunk: [P, FT*K*nt] (f = ft*128 + p)
            if c == 0:
                xt = xt0
            else:
                xt = xpool.tile([P, FT * KN], f16, tag="xc")
                nc.sync.dma_start(
                    out=xt[:],
                    in_=_rap(
                        xr_b,
                        c * F * KN,
                        [[KN, P], [P * KN, FT], [1, KN]],
                    ),
                )
            if c == 0:
                load_aux(0, 0)
            for gt in range(FT):
                xo = gt * KN
                dc = dcpool.tile([P, K, nt], f16, tag="dc")
                cur = {
                    "dc": dc, "xt": xt[:], "xo": xo, "c": c, "gt": gt,
                    "ax": _ap(axh[(c, gt // 2)][:], (gt % 2) * 4 * nt,
                              [[1, 4 * nt]]),
                }

                # dcB of the PREVIOUS iter: ScalarE head normally; for
                # iter 0's dcB the DVE is still idle and 2x-faster
                if prev is not None:
                    if prev2 is None:
                        nc.vector.tensor_copy(
                            out=prev["dc"][:, KH:K, :], in_=prev["pdB"][:]
                        )
                    else:
                        nc.scalar.copy(out=prev["dc"][:, KH:K, :], in_=prev["pdB"][:])
                # ---- matmul halves -> PSUM ----
                pds = []
                for half, pool in ((0, papool), (1, pbpool)):
                    pd = pool.tile([P, KH, nt], f32, tag=("pda", "pdb")[half])
                    # Dummy matmul absorbs the PSUM-slot-release wait
                    # (1-wait limit on Matmult structs).
                    nc.tensor.matmul(
                        pd[:, 0, 0:1], wsb[0][:, 0:P], wsb[0][:, 0:1],
                        start=True, stop=True,
                    )
                    k0 = half * KH
                    for ft in range(FT):
                        for jj in range(KH):
                            nc.tensor.matmul(
                                pd[:, jj, :],
                                wsb[ft][:, gt * P : (gt + 1) * P],
                                _ap(xt[:], ft * KN + (k0 + jj) * nt, [[1, nt]]),
                                start=(ft == 0),
                                stop=(ft == FT - 1),
                            )
                    pds.append(pd)
                    if half == 0:
                        if prev is None and prev2 is None:
                            # iter 0: the DVE is idle during fill -- do the
                            # PSUM->SBUF copy there (2x, no cross-engine
                            # handoff) and chase it with products A (which
                            # need only dc half A)
                            nc.vector.tensor_copy(out=dc[:, 0:KH, :], in_=pd[:])
                            emit_products(cur, part=0)
                        else:
                            nc.scalar.copy(out=dc[:, 0:KH, :], in_=pd[:])
                cur["pdB"] = pds[1]

                # ---- elementwise streams (DVE order = emission order) ----
                if prev is not None:
                    emit_products(prev, part=2 if "p" not in prev else 1)
                    emit_t1(prev)
                if gt == 0:
                    load_aux(c, 1)
                elif gt == 2 and c + 1 < nch:
                    load_aux(c + 1, 0)
                if prev3 is not None:
                    emit_og_o2(prev3)
                if prev2 is not None:
                    emit_t2_kf(prev2)
                    emit_gate(prev2)
                if prev3 is not None:
                    emit_out(prev3)

                prev3 = prev2
                prev2 = prev
                prev = cur

        # ---- drain ----
        nc.scalar.copy(out=prev["dc"][:, KH:K, :], in_=prev["pdB"][:])
        emit_products(prev, part=1 if "p" in prev else 2)
        emit_t1(prev)
        emit_og_o2(prev3)
        emit_t2_kf(prev2)
        emit_gate(prev2)
        emit_out(prev3)
        emit_og_o2(prev2)
        emit_t2_kf(prev)
        emit_gate(prev)
        emit_out(prev2)
        emit_og_o2(prev)
        emit_out(prev)

    _split_waits(nc)
    return nc


# Engine datapath structs (Matmult/TT/STT/Act/...) only carry ONE sync wait on
# TRN2 walrus; sequencer instructions (NoOp) can each carry one more.  Hoist
# surplus waits onto same-engine NoOps placed just before the instruction.
def _split_waits(nc):
    nnop = 0
    for fn in nc.m.functions:
        for blk in fn.blocks:
            out = []
            for inst in blk.instructions:
                si = inst.sync_info
                if si is not None and si.on_wait and len(si.on_wait) > 1:
                    for w in si.on_wait[:-1]:
                        nop = mybir.InstNoOp(
                            name=f"{inst.name}-sw{nnop}",
                            opcode="NoOp",
                            engine=inst.engine,
                            sync_info=mybir.SyncInfo(on_wait=[w], on_update=[]),
                        )
                        nnop += 1
                        out.append(nop)
                    inst.sync_info = mybir.SyncInfo(
                        on_wait=[si.on_wait[-1]], on_update=list(si.on_update)
                    )
                out.append(inst)
            blk.instructions[:] = out
    return nc


_NC_CACHE = {}

NCH, NT = N // 512, 512


def _get_nc(n_total=N, nt=NT):
    key = (n_total, nt)
    if key not in _NC_CACHE:
        _NC_CACHE[key] = build_nc(n_total, nt)
    return _NC_CACHE[key]


def _to_f16(a: np.ndarray) -> np.ndarray:
    return np.ascontiguousarray(a.astype(np.float16))


def _chunked(a: np.ndarray) -> np.ndarray:
    # [F, C, N] -> chunk-contiguous [nch, F, C, nt]
    Fd, Cd, Nd = a.shape
    return np.ascontiguousarray(
        a.reshape(Fd, Cd, NCH, NT).transpose(2, 0, 1, 3)
    )


# x k-planes stored on-device in this order; matmul/products/out are all
# slot-relative, the host un-permutes planes on the way out (free).
TAU = [1, 3, 5, 7, 2, 6, 0, 4]
INV = [6, 0, 4, 1, 7, 2, 5, 3]  # INV[k] = slot of true plane k


def make_in_maps(x: np.ndarray, W: np.ndarray):
    wt = _to_f16(W.T.copy())
    x16 = _to_f16(x[:, :, TAU, :])
    xa = np.stack(
        [
            x[:, :, 6, :],
            x[:, :, 2, :],
            2.0 * x[:, :, 0, :] - x[:, :, 4, :],
            2.0 * x[:, :, 4, :] - x[:, :, 0, :],
        ],
        axis=2,
    )
    xa16 = _to_f16(xa)
    return [
        {"xr": _chunked(x16[b]), "wt": wt, "auxr": _chunked(xa16[b])}
        for b in range(B)
    ]


def post(res) -> np.ndarray:
    # gather + un-chunk: [nch, F, K, nt] -> [F, K, N]
    return np.stack(
        [
            res.results[b]["outr"]
            .transpose(1, 2, 0, 3)[:, INV]
            .reshape(F, K, N)
            .astype(np.float32)
            for b in range(B)
        ],
        axis=0,
    )


def kernel(x: np.ndarray, W: np.ndarray) -> np.ndarray:
    assert x.shape == (B, F, K, N) and W.shape == (F, F)
    in_maps = make_in_maps(x, W)
    nc = _get_nc()
    res = run_bass_kernel_spmd(nc, in_maps, list(range(B)))
    return post(res)


if __name__ == "__main__":
    xs = np.random.randn(B, F, K, N).astype(np.float32)
    Ws = (np.random.randn(F, F) / np.sqrt(F)).astype(np.float32)
    o = kernel(xs, Ws)
    print(o.shape, o.dtype)
